# revision 7
# baseline (speedup 1.0000x reference)
"""Bass/Tile TRN2 kernel for a batched self-attention layer.

Reference computation (per batch b, N = 64*64 = 4096 tokens, C = 256, Dp = 32):
    f = input_h @ f_w          [N, Dp]
    g = x @ g_w                [N, Dp]
    s = g @ f.T                [N, N]
    beta = softmax(s, -1)
    o = beta @ input_h         [N, C]
    out = concat([o, x], -1)   [N, 2C]

Sharding: 8 cores = (batch b, query-half) pairs. Each core handles 2048 query
rows of one batch with the full 4096-key attention for that batch.

Per-core kernel strategy:
  * Compute fT [Dp, 4096] and gT [Dp, 2048] (d on partitions) via PE
    transposes of h / x (fp16) plus small fp16 matmuls against the weights.
  * Main loop over (query-block of 512) x (key-chunk of 128):
      - QK^T directly in TRANSPOSED layout: sT[k, q] = fT_chunk.T @ gT_block
        (fp16 matmul, contraction over Dp=32). exp(sT) is then directly the
        stationary operand of the PV matmul -- no transposes of beta needed.
      - exp on the scalar engine, PSUM -> SBUF as float32r (fp32 range --
        logits reach 30, so no 16-bit float can hold exp(s); and softmax
        needs no max-subtraction since exp(30) is well inside fp32).
      - PV: o[q, :] accumulates exp_chunk.T @ hR_chunk over the 32 key
        chunks as a float32r matmul (full PE speed at free-dim >= 256).
        hR tiles carry an appended ones-column, so column C of the PSUM
        accumulator is the softmax denominator for free.
  * Normalize with DVE reciprocal + ACT scale-multiply, DMA out.

float32r matmuls are self-loading single instructions that can carry only ONE
semaphore wait in walrus codegen. All PV dependencies (exp output, hR tiles,
o-accumulator slot recycling, prior PSUM-pool releases) are therefore funneled
through the scalar engine, so every PV matmul needs at most one ACT wait.
fp16 matmuls lower to LDWEIGHTS+MATMUL and can take multiple waits, so the
rest of the kernel has no such constraint.

The host-side wrapper shards inputs, runs the SPMD kernel on 8 cores, and
re-assembles the full [4, 64, 64, 512] output (the concat with x is pure data
movement, done on the host).
"""

import numpy as np

import concourse.bass as bass
import concourse.tile as tile
from concourse import bacc
from concourse import mybir
from concourse.bass_utils import run_bass_kernel_spmd
from concourse.masks import make_identity

F32 = mybir.dt.float32
F32R = mybir.dt.float32r
F16 = mybir.dt.float16

B, W, C, D = 4, 64, 256, 32
N = W * W                 # 4096 tokens (keys) per batch
NCORES = 8
SHARDS_PER_BATCH = NCORES // B   # 2
NQ = N // SHARDS_PER_BATCH       # 2048 query rows per core
KC = 128                         # key chunk (PE partition dim)
NKC = N // KC                    # 32 key chunks
QBLK = 512                       # query block (moving free dim)
NQB = NQ // QBLK                 # 4 query blocks per core
QSUB = 128                       # query sub-tile (PV stationary M)
NQSUB = QBLK // QSUB             # 4
Exp = mybir.ActivationFunctionType.Exp


def _build() -> bass.Bass:
    nc = bacc.Bacc("TRN2", target_bir_lowering=False)

    xs = nc.declare_dram_parameter("xs", [NQ, C], F32, isOutput=False)
    h = nc.declare_dram_parameter("h", [N, C], F32, isOutput=False)
    fw = nc.declare_dram_parameter("fw", [C, D], F32, isOutput=False)
    gw = nc.declare_dram_parameter("gw", [C, D], F32, isOutput=False)
    o = nc.declare_dram_parameter("o", [NQ, C], F32, isOutput=True)

    with tile.TileContext(nc) as tc:
        with (
            tc.tile_pool(name="const", bufs=1) as const_pool,
            tc.tile_pool(name="hr", bufs=1) as hr_pool,
            tc.tile_pool(name="stage", bufs=6) as stage_pool,
            tc.tile_pool(name="conv16", bufs=6) as c16_pool,
            tc.tile_pool(name="proj", bufs=1) as proj_pool,
        ):
            identf = const_pool.tile([128, 128], F32)
            make_identity(nc, identf)
            ident = const_pool.tile([128, 128], F16)
            nc.vector.tensor_copy(ident[:, :], identf[:, :])

            zbias = const_pool.tile([128, 1], F32)
            nc.vector.memset(zbias[:, :], 0.0)

            fwg_st = const_pool.tile([128, 2, 2 * D], F32)
            for cc in range(2):
                nc.sync.dma_start(out=fwg_st[:, cc, 0:D], in_=fw[cc * 128:(cc + 1) * 128, :])
                nc.sync.dma_start(out=fwg_st[:, cc, D:2 * D], in_=gw[cc * 128:(cc + 1) * 128, :])
            fwg16 = const_pool.tile([128, 2, 2 * D], F16)
            nc.vector.tensor_copy(fwg16[:, :, :], fwg_st[:, :, :])

            # hR tiles: [128 keys, C + ones column] float32r, resident, PV rhs.
            # Produced by ACT copies so PV's dependency is on the scalar engine.
            hr_tiles = []
            for t in range(NKC):
                hr = hr_pool.tile([128, C + 2], F32R, tag=f"hr{t}", name=f"hr{t}")
                hr_tiles.append(hr)

            # fT [Dp, N] and gT [Dp, NQ] in fp16, d on partitions.
            fT_sb = proj_pool.tile([D, NKC // 4, 512], F16)
            gT_sb = proj_pool.tile([D, NQB, 512], F16)

            with (
                tc.tile_pool(name="tps", bufs=4, space="PSUM") as tps_pool,
                tc.tile_pool(name="fgps", bufs=2, space="PSUM") as fg_pool,
                tc.tile_pool(name="tsb", bufs=3) as tsb_pool,
            ):
                # gT from x: DMA f32 staging -> DVE fp16 convert -> PE transpose
                # (c on partitions) -> ACT copy to SBUF -> fp16 matmul with gw.
                for qb in range(NQB):
                    xT = tsb_pool.tile([128, 2, 512], F16, tag="xT", name=f"xT{qb}")
                    for j in range(4):
                        r0 = (qb * 4 + j) * 128
                        xst = stage_pool.tile([128, C], F32, tag="xst", name=f"xst{qb}_{j}")
                        nc.sync.dma_start(out=xst[:, :], in_=xs[r0:r0 + 128, :])
                        x16 = c16_pool.tile([128, C], F16, tag="x16", name=f"x16{qb}_{j}")
                        nc.vector.tensor_copy(x16[:, :], xst[:, :])
                        for cc in range(2):
                            ps = tps_pool.tile([128, 128], F16, tag="tps", name=f"psx{qb}_{j}_{cc}")
                            nc.tensor.transpose(ps[:, :], x16[:, cc * 128:(cc + 1) * 128], ident[:, :])
                            nc.scalar.copy(xT[:, cc, j * 128:(j + 1) * 128], ps[:, :])
                    g_ps = fg_pool.tile([D, 512], F32, tag="fg", name=f"gps{qb}")
                    for cc in range(2):
                        nc.tensor.matmul(
                            g_ps[:, :],
                            fwg16[:, cc, D:2 * D],
                            xT[:, cc, :],
                            start=(cc == 0),
                            stop=(cc == 1),
                        )
                    nc.scalar.copy(gT_sb[:, qb, :], g_ps[:, :])

                # fT from h, same pattern over 8 blocks; also produce the
                # float32r hR tiles (ACT copy of the f32 staging + ones col).
                for p in range(NKC // 4):
                    hT = tsb_pool.tile([128, 2, 512], F16, tag="hT", name=f"hT{p}")
                    for j in range(4):
                        t = p * 4 + j
                        hst = stage_pool.tile([128, C + 2], F32, tag="hst", name=f"hst{t}")
                        nc.sync.dma_start(out=hst[:, 0:C], in_=h[t * 128:(t + 1) * 128, :])
                        nc.vector.memset(hst[:, C:C + 1], 1.0)
                        nc.vector.memset(hst[:, C + 1:C + 2], 0.0)
                        nc.scalar.copy(hr_tiles[t][:, :], hst[:, :].bitcast(F32R))
                        h16 = c16_pool.tile([128, C], F16, tag="h16", name=f"h16{t}")
                        nc.vector.tensor_copy(h16[:, :], hst[:, 0:C])
                        for cc in range(2):
                            ps = tps_pool.tile([128, 128], F16, tag="tps", name=f"psh{t}_{cc}")
                            nc.tensor.transpose(ps[:, :], h16[:, cc * 128:(cc + 1) * 128], ident[:, :])
                            nc.scalar.copy(hT[:, cc, j * 128:(j + 1) * 128], ps[:, :])
                    f_ps = fg_pool.tile([D, 512], F32, tag="fg", name=f"fps{p}")
                    for cc in range(2):
                        nc.tensor.matmul(
                            f_ps[:, :],
                            fwg16[:, cc, 0:D],
                            hT[:, cc, :],
                            start=(cc == 0),
                            stop=(cc == 1),
                        )
                    nc.scalar.copy(fT_sb[:, p, :], f_ps[:, :])

            # Main attention loop.
            with (
                tc.tile_pool(name="sps", bufs=3, space="PSUM") as s_pool,
                tc.tile_pool(name="ops", bufs=1, space="PSUM") as o_pool,
                tc.tile_pool(name="esb", bufs=4) as e_pool,
                tc.tile_pool(name="osb", bufs=4) as out_pool,
                tc.tile_pool(name="rsb", bufs=4) as r_pool,
            ):
                for qb in range(NQB):
                    o_ps = [
                        o_pool.tile([128, C + 2], F32, tag=f"o{i}", name=f"ops{qb}_{i}")
                        for i in range(NQSUB)
                    ]
                    for k in range(NKC):
                        # sT chunk [128 keys, 512 queries] = fT_chunk.T @ gT_block
                        s_ps = s_pool.tile([128, QBLK], F32, tag="s", name=f"sps{qb}_{k}")
                        nc.tensor.matmul(
                            s_ps[:, :],
                            fT_sb[:, k // 4, (k % 4) * 128:(k % 4 + 1) * 128],
                            gT_sb[:, qb, :],
                            start=True,
                            stop=True,
                        )
                        e_sb = e_pool.tile([128, QBLK], F32R, tag="e", name=f"e{qb}_{k}")
                        nc.scalar.activation(e_sb[:, :], s_ps[:, :], Exp, bias=zbias[:, :])
                        for i in range(NQSUB):
                            nc.tensor.matmul(
                                o_ps[i][:, :],
                                e_sb[:, i * 128:(i + 1) * 128],
                                hr_tiles[k][:, :],
                                start=(k == 0),
                                stop=(k == NKC - 1),
                            )
                    for i in range(NQSUB):
                        den = r_pool.tile([128, 1], F32, tag="den", name=f"den{qb}_{i}")
                        nc.scalar.copy(den[:, :], o_ps[i][:, C:C + 1])
                        rec = r_pool.tile([128, 1], F32, tag="rec", name=f"rec{qb}_{i}")
                        nc.vector.reciprocal(rec[:, :], den[:, :])
                        out_sb = out_pool.tile([128, C], F32, tag="ob", name=f"ob{qb}_{i}")
                        nc.scalar.mul(out_sb[:, :], o_ps[i][:, 0:C], rec[:, :])
                        r0 = qb * QBLK + i * 128
                        nc.sync.dma_start(out=o[r0:r0 + 128, :], in_=out_sb[:, :])

    nc.finalize()
    return nc


_CACHE: dict = {}


def _get_nc() -> bass.Bass:
    if "nc" not in _CACHE:
        _CACHE["nc"] = _build()
    return _CACHE["nc"]


def _shard(x, input_h, f_w, g_w):
    xf = np.ascontiguousarray(np.asarray(x, dtype=np.float32).reshape(B, N, C))
    hf = np.ascontiguousarray(np.asarray(input_h, dtype=np.float32).reshape(B, N, C))
    fwf = np.ascontiguousarray(np.asarray(f_w, dtype=np.float32).reshape(C, D))
    gwf = np.ascontiguousarray(np.asarray(g_w, dtype=np.float32).reshape(C, D))
    in_maps = []
    for c in range(NCORES):
        b, half = divmod(c, SHARDS_PER_BATCH)
        in_maps.append(
            {
                "xs": np.ascontiguousarray(xf[b, half * NQ:(half + 1) * NQ]),
                "h": hf[b],
                "fw": fwf,
                "gw": gwf,
            }
        )
    return in_maps


def _gather(results, x):
    of = np.empty((B, N, C), np.float32)
    for c in range(NCORES):
        b, half = divmod(c, SHARDS_PER_BATCH)
        of[b, half * NQ:(half + 1) * NQ] = results[c]["o"]
    o4 = of.reshape(B, W, W, C)
    x4 = np.asarray(x, dtype=np.float32).reshape(B, W, W, C)
    return np.concatenate([o4, x4], axis=-1)


def run(inputs: dict, trace: bool = False):
    """Run the kernel; returns (full_output, BassKernelResults)."""
    in_maps = _shard(**inputs)
    res = run_bass_kernel_spmd(_get_nc(), in_maps, list(range(NCORES)), trace=trace)
    out = _gather(res.results, inputs["x"])
    return out, res


def kernel(**inputs) -> np.ndarray:
    out, _ = run(inputs, trace=False)
    return out


# revision 8
# speedup vs baseline: 1.2014x; 1.2014x over previous
"""Bass/Tile TRN2 kernel for a batched self-attention layer.

Reference computation (per batch b, N = 64*64 = 4096 tokens, C = 256, Dp = 32):
    f = input_h @ f_w          [N, Dp]
    g = x @ g_w                [N, Dp]
    s = g @ f.T                [N, N]
    beta = softmax(s, -1)
    o = beta @ input_h         [N, C]
    out = concat([o, x], -1)   [N, 2C]

Sharding: 8 cores = (batch b, query-half) pairs. Each core handles 2048 query
rows of one batch with the full 4096-key attention for that batch.

Per-core kernel strategy:
  * Compute fT [Dp, 4096] and gT [Dp, 2048] (d on partitions) via PE
    transposes of h / x (fp16) plus small fp16 matmuls against the weights.
  * Main loop over (query-block of 512) x (key-chunk of 128):
      - QK^T directly in TRANSPOSED layout: sT[k, q] = fT_chunk.T @ gT_block
        (fp16 matmul, contraction over Dp=32). exp(sT) is then directly the
        stationary operand of the PV matmul -- no transposes of beta needed.
      - exp on the scalar engine, PSUM -> SBUF as float32r (fp32 range --
        logits reach 30, so no 16-bit float can hold exp(s); and softmax
        needs no max-subtraction since exp(30) is well inside fp32).
      - PV: o[q, :] accumulates exp_chunk.T @ hR_chunk over the 32 key
        chunks as a float32r matmul (full PE speed at free-dim >= 256).
        hR tiles carry an appended ones-column, so column C of the PSUM
        accumulator is the softmax denominator for free.
  * Normalize with DVE reciprocal + ACT scale-multiply, DMA out.

float32r matmuls are self-loading single instructions that can carry only ONE
semaphore wait in walrus codegen. All PV dependencies (exp output, hR tiles,
o-accumulator slot recycling, prior PSUM-pool releases) are therefore funneled
through the scalar engine, so every PV matmul needs at most one ACT wait.
fp16 matmuls lower to LDWEIGHTS+MATMUL and can take multiple waits, so the
rest of the kernel has no such constraint.

The host-side wrapper shards inputs, runs the SPMD kernel on 8 cores, and
re-assembles the full [4, 64, 64, 512] output (the concat with x is pure data
movement, done on the host).
"""

import numpy as np

import concourse.bass as bass
import concourse.tile as tile
from concourse import bacc
from concourse import mybir
from concourse.bass_utils import run_bass_kernel_spmd
from concourse.masks import make_identity

F32 = mybir.dt.float32
F32R = mybir.dt.float32r
F16 = mybir.dt.float16
BF16 = mybir.dt.bfloat16

B, W, C, D = 4, 64, 256, 32
N = W * W                 # 4096 tokens (keys) per batch
NCORES = 8
SHARDS_PER_BATCH = NCORES // B   # 2
NQ = N // SHARDS_PER_BATCH       # 2048 query rows per core
KC = 128                         # key chunk (PE partition dim)
NKC = N // KC                    # 32 key chunks
QBLK = 512                       # query block (moving free dim)
NQB = NQ // QBLK                 # 4 query blocks per core
QSUB = 128                       # query sub-tile (PV stationary M)
NQSUB = QBLK // QSUB             # 4
Exp = mybir.ActivationFunctionType.Exp


def _build() -> bass.Bass:
    nc = bacc.Bacc("TRN2", target_bir_lowering=False)

    xs = nc.declare_dram_parameter("xs", [NQ, C], F32, isOutput=False)
    h = nc.declare_dram_parameter("h", [N, C], F32, isOutput=False)
    fw = nc.declare_dram_parameter("fw", [C, D], F32, isOutput=False)
    gw = nc.declare_dram_parameter("gw", [C, D], F32, isOutput=False)
    o = nc.declare_dram_parameter("o", [NQ, C], F32, isOutput=True)

    with tile.TileContext(nc) as tc:
        with (
            tc.tile_pool(name="const", bufs=1) as const_pool,
            tc.tile_pool(name="hr", bufs=1) as hr_pool,
            tc.tile_pool(name="stage", bufs=6) as stage_pool,
            tc.tile_pool(name="conv16", bufs=6) as c16_pool,
            tc.tile_pool(name="proj", bufs=1) as proj_pool,
        ):
            identf = const_pool.tile([128, 128], F32)
            make_identity(nc, identf)
            ident = const_pool.tile([128, 128], F16)
            nc.vector.tensor_copy(ident[:, :], identf[:, :])

            zbias = const_pool.tile([128, 1], F32)
            nc.vector.memset(zbias[:, :], 0.0)

            fwg_st = const_pool.tile([128, 2, 2 * D], F32)
            for cc in range(2):
                nc.sync.dma_start(out=fwg_st[:, cc, 0:D], in_=fw[cc * 128:(cc + 1) * 128, :])
                nc.sync.dma_start(out=fwg_st[:, cc, D:2 * D], in_=gw[cc * 128:(cc + 1) * 128, :])
            fwg16 = const_pool.tile([128, 2, 2 * D], F16)
            nc.vector.tensor_copy(fwg16[:, :, :], fwg_st[:, :, :])

            # hR tiles: [128 keys, C + ones column] float32r, resident, PV rhs.
            # Produced by ACT copies so PV's dependency is on the scalar engine.
            hr_tiles = []
            for t in range(NKC):
                hr = hr_pool.tile([128, C + 2], BF16, tag=f"hr{t}", name=f"hr{t}")
                hr_tiles.append(hr)

            # fT [Dp, N] and gT [Dp, NQ] in fp16, d on partitions.
            fT_sb = proj_pool.tile([D, NKC // 4, 512], F16)
            gT_sb = proj_pool.tile([D, NQB, 512], F16)

            with (
                tc.tile_pool(name="tps", bufs=4, space="PSUM") as tps_pool,
                tc.tile_pool(name="fgps", bufs=2, space="PSUM") as fg_pool,
                tc.tile_pool(name="tsb", bufs=3) as tsb_pool,
            ):
                # gT from x: DMA f32 staging -> DVE fp16 convert -> PE transpose
                # (c on partitions) -> ACT copy to SBUF -> fp16 matmul with gw.
                for qb in range(NQB):
                    xT = tsb_pool.tile([128, 2, 512], F16, tag="xT", name=f"xT{qb}")
                    for j in range(4):
                        r0 = (qb * 4 + j) * 128
                        xst = stage_pool.tile([128, C], F32, tag="xst", name=f"xst{qb}_{j}")
                        nc.sync.dma_start(out=xst[:, :], in_=xs[r0:r0 + 128, :])
                        x16 = c16_pool.tile([128, C], F16, tag="x16", name=f"x16{qb}_{j}")
                        nc.vector.tensor_copy(x16[:, :], xst[:, :])
                        for cc in range(2):
                            ps = tps_pool.tile([128, 128], F16, tag="tps", name=f"psx{qb}_{j}_{cc}")
                            nc.tensor.transpose(ps[:, :], x16[:, cc * 128:(cc + 1) * 128], ident[:, :])
                            nc.vector.tensor_copy(xT[:, cc, j * 128:(j + 1) * 128], ps[:, :])
                    g_ps = fg_pool.tile([D, 512], F32, tag="fg", name=f"gps{qb}")
                    for cc in range(2):
                        nc.tensor.matmul(
                            g_ps[:, :],
                            fwg16[:, cc, D:2 * D],
                            xT[:, cc, :],
                            start=(cc == 0),
                            stop=(cc == 1),
                        )
                    nc.vector.tensor_copy(gT_sb[:, qb, :], g_ps[:, :])

                # fT from h, same pattern over 8 blocks; also produce the
                # float32r hR tiles (ACT copy of the f32 staging + ones col).
                for p in range(NKC // 4):
                    hT = tsb_pool.tile([128, 2, 512], F16, tag="hT", name=f"hT{p}")
                    for j in range(4):
                        t = p * 4 + j
                        hst = stage_pool.tile([128, C + 2], F32, tag="hst", name=f"hst{t}")
                        nc.sync.dma_start(out=hst[:, 0:C], in_=h[t * 128:(t + 1) * 128, :])
                        nc.vector.memset(hst[:, C:C + 1], 1.0)
                        nc.vector.memset(hst[:, C + 1:C + 2], 0.0)
                        nc.vector.tensor_copy(hr_tiles[t][:, :], hst[:, :])
                        h16 = c16_pool.tile([128, C], F16, tag="h16", name=f"h16{t}")
                        nc.vector.tensor_copy(h16[:, :], hst[:, 0:C])
                        for cc in range(2):
                            ps = tps_pool.tile([128, 128], F16, tag="tps", name=f"psh{t}_{cc}")
                            nc.tensor.transpose(ps[:, :], h16[:, cc * 128:(cc + 1) * 128], ident[:, :])
                            nc.vector.tensor_copy(hT[:, cc, j * 128:(j + 1) * 128], ps[:, :])
                    f_ps = fg_pool.tile([D, 512], F32, tag="fg", name=f"fps{p}")
                    for cc in range(2):
                        nc.tensor.matmul(
                            f_ps[:, :],
                            fwg16[:, cc, 0:D],
                            hT[:, cc, :],
                            start=(cc == 0),
                            stop=(cc == 1),
                        )
                    nc.vector.tensor_copy(fT_sb[:, p, :], f_ps[:, :])

            # Main attention loop.
            with (
                tc.tile_pool(name="sps", bufs=3, space="PSUM") as s_pool,
                tc.tile_pool(name="ops", bufs=1, space="PSUM") as o_pool,
                tc.tile_pool(name="esb", bufs=4) as e_pool,
                tc.tile_pool(name="osb", bufs=4) as out_pool,
                tc.tile_pool(name="rsb", bufs=4) as r_pool,
            ):
                for qb in range(NQB):
                    o_ps = [
                        o_pool.tile([128, C + 2], F32, tag=f"o{i}", name=f"ops{qb}_{i}")
                        for i in range(NQSUB)
                    ]
                    for k in range(NKC):
                        # sT chunk [128 keys, 512 queries] = fT_chunk.T @ gT_block
                        s_ps = s_pool.tile([128, QBLK], F32, tag="s", name=f"sps{qb}_{k}")
                        nc.tensor.matmul(
                            s_ps[:, :],
                            fT_sb[:, k // 4, (k % 4) * 128:(k % 4 + 1) * 128],
                            gT_sb[:, qb, :],
                            start=True,
                            stop=True,
                        )
                        e_sb = e_pool.tile([128, QBLK], BF16, tag="e", name=f"e{qb}_{k}")
                        nc.scalar.activation(e_sb[:, :], s_ps[:, :], Exp, bias=zbias[:, :])
                        for i in range(NQSUB):
                            nc.tensor.matmul(
                                o_ps[i][:, :],
                                e_sb[:, i * 128:(i + 1) * 128],
                                hr_tiles[k][:, :],
                                start=(k == 0),
                                stop=(k == NKC - 1),
                            )
                    for i in range(NQSUB):
                        rec = r_pool.tile([128, 1], F32, tag="rec", name=f"rec{qb}_{i}")
                        nc.vector.reciprocal(rec[:, :], o_ps[i][:, C:C + 1])
                        out_sb = out_pool.tile([128, C], F32, tag="ob", name=f"ob{qb}_{i}")
                        nc.vector.tensor_scalar_mul(out_sb[:, :], o_ps[i][:, 0:C], rec[:, :])
                        r0 = qb * QBLK + i * 128
                        nc.sync.dma_start(out=o[r0:r0 + 128, :], in_=out_sb[:, :])

    nc.finalize()
    return nc


_CACHE: dict = {}


def _get_nc() -> bass.Bass:
    if "nc" not in _CACHE:
        _CACHE["nc"] = _build()
    return _CACHE["nc"]


def _shard(x, input_h, f_w, g_w):
    xf = np.ascontiguousarray(np.asarray(x, dtype=np.float32).reshape(B, N, C))
    hf = np.ascontiguousarray(np.asarray(input_h, dtype=np.float32).reshape(B, N, C))
    fwf = np.ascontiguousarray(np.asarray(f_w, dtype=np.float32).reshape(C, D))
    gwf = np.ascontiguousarray(np.asarray(g_w, dtype=np.float32).reshape(C, D))
    in_maps = []
    for c in range(NCORES):
        b, half = divmod(c, SHARDS_PER_BATCH)
        in_maps.append(
            {
                "xs": np.ascontiguousarray(xf[b, half * NQ:(half + 1) * NQ]),
                "h": hf[b],
                "fw": fwf,
                "gw": gwf,
            }
        )
    return in_maps


def _gather(results, x):
    of = np.empty((B, N, C), np.float32)
    for c in range(NCORES):
        b, half = divmod(c, SHARDS_PER_BATCH)
        of[b, half * NQ:(half + 1) * NQ] = results[c]["o"]
    o4 = of.reshape(B, W, W, C)
    x4 = np.asarray(x, dtype=np.float32).reshape(B, W, W, C)
    return np.concatenate([o4, x4], axis=-1)


def run(inputs: dict, trace: bool = False):
    """Run the kernel; returns (full_output, BassKernelResults)."""
    in_maps = _shard(**inputs)
    res = run_bass_kernel_spmd(_get_nc(), in_maps, list(range(NCORES)), trace=trace)
    out = _gather(res.results, inputs["x"])
    return out, res


def kernel(**inputs) -> np.ndarray:
    out, _ = run(inputs, trace=False)
    return out
